# revision 7
# baseline (speedup 1.0000x reference)
"""Trainium2 Bass kernel for nn_CachePolicyModel (cache policy: LSTM step +
bilinear attention over history + scorer/reuse heads with masked renormalize).

Data-parallel over batch: B=64 split as 8 batches on each of 8 NeuronCores.

Math restructuring vs the reference (exact up to fp reassociation):
  - A_b = attn_W @ hist_b^T  (D_ID x T) precomputed per batch, so
    scores_b^T costs L*D*T instead of L*(D+T)*H.
  - contexts (L x 384) are never materialized: line_score_l = scorer_W . ctx_l
    = sum_t w_lt * (scorer_W . values_t), so values are projected onto the two
    head vectors first (sv_t, rv_t) and exp(scores) is reduced against
    [sv | rv | 1] with one PE matmul per batch -> numerators + denominator.
  - softmax-over-L then mask+renormalize == masked softmax over L
    (the dense-softmax denominator and scorer_b cancel exactly).

Gather strategy: obj_id_table is split into two 32768-row halves (so row
indices fit int16 for gpsimd.dma_gather), cast to bf16, and each half gets
one host-chosen never-referenced row zeroed as the "dummy" target for
indices that live in the other half. Two transposing dma_gathers (8192
indices each, xbar-transposed during the DMA) deliver q^T for all 8 local
batches directly; the lo/hi merge happens for free by accumulating both
halves' matmuls into the same PSUM bank.
"""

import os
import sys

import ml_dtypes
import numpy as np

for _p in ("/opt/trn_rl_repo", os.path.expanduser("~/.axon_site/_ro/trn_rl_repo")):
    if os.path.isdir(_p) and _p not in sys.path:
        sys.path.append(_p)

import concourse.bacc as bacc
import concourse.bass as bass
import concourse.mybir as mybir
import concourse.tile as tile
from concourse.bass import IndirectOffsetOnAxis
from concourse.bass_utils import run_bass_kernel_spmd

F32 = mybir.dt.float32
I32 = mybir.dt.int32
I16 = mybir.dt.int16
BF16 = mybir.dt.bfloat16
BF16_NP = ml_dtypes.bfloat16

# Problem shape (hardcoded per contract)
B, L, H, T, D_ID, D_SZ, D_POS = 64, 1024, 256, 128, 128, 64, 128
V_ID, V_SZ = 65536, 256
VH = V_ID // 2  # 32768 rows per half-table
NCORES = 8
BC = B // NCORES  # batches per core = 8
NIDX = BC * L  # 8192 gather indices per core
KIN = D_ID + D_SZ + H  # 448, stacked LSTM contraction dim


def build_program():
    """Build the per-core Bass/Tile program (identical on all cores)."""
    nc = bacc.Bacc("TRN2", target_bir_lowering=False, debug=False)

    f = F32
    # --- DRAM I/O ---
    t_lo = nc.dram_tensor("t_lo", [VH, D_ID], BF16, kind="ExternalInput").ap()
    t_hi = nc.dram_tensor("t_hi", [VH, D_ID], BF16, kind="ExternalInput").ap()
    t_sz = nc.dram_tensor("t_sz", [V_SZ, D_SZ], f, kind="ExternalInput").ap()
    idx_lo_d = nc.dram_tensor("idx_lo", [128, NIDX // 16], I16, kind="ExternalInput").ap()
    idx_hi_d = nc.dram_tensor("idx_hi", [128, NIDX // 16], I16, kind="ExternalInput").ap()
    w_all_d = nc.dram_tensor("w_all", [KIN, 4 * H], f, kind="ExternalInput").ap()
    bias8_d = nc.dram_tensor("bias8", [128, 8], f, kind="ExternalInput").ap()
    attn_wt_d = nc.dram_tensor("attn_wt", [H, D_ID], BF16, kind="ExternalInput").ap()
    post_d = nc.dram_tensor("post", [D_POS, T], BF16, kind="ExternalInput").ap()
    w_heads_d = nc.dram_tensor("w_heads", [3, 128, 2], BF16, kind="ExternalInput").ap()
    obj_lo_d = nc.dram_tensor("obj_lo", [BC, 1], I32, kind="ExternalInput").ap()
    obj_hi_d = nc.dram_tensor("obj_hi", [BC, 1], I32, kind="ExternalInput").ap()
    size_idx_d = nc.dram_tensor("size_idx", [BC, 1], I32, kind="ExternalInput").ap()
    lengths_d = nc.dram_tensor("lengths_f", [BC, 1], f, kind="ExternalInput").ap()
    c0t_d = nc.dram_tensor("c0t", [H, BC], f, kind="ExternalInput").ap()
    h0t_d = nc.dram_tensor("h0t", [H, BC], f, kind="ExternalInput").ap()
    histt_d = nc.dram_tensor("histt", [H, BC, T - 1], BF16, kind="ExternalInput").ap()
    rbias_d = nc.dram_tensor("rbias", [BC, 1], f, kind="ExternalInput").ap()

    out_probs = nc.dram_tensor("out_probs", [BC, L], f, kind="ExternalOutput").ap()
    out_reuse = nc.dram_tensor("out_reuse", [BC, L], f, kind="ExternalOutput").ap()

    # --- NEFF-embedded constants ---
    ident_d = nc.inline_tensor(np.eye(16, dtype=np.float32), "ident_c").ap()
    iota_d = nc.inline_tensor(
        np.broadcast_to(np.arange(L, dtype=np.float32), (BC, L)).copy(), "iota_c"
    ).ap()

    AF = mybir.ActivationFunctionType
    OP = mybir.AluOpType

    with tile.TileContext(nc) as tc:
        with (
            tc.tile_pool(name="const", bufs=1) as cpool,
            tc.tile_pool(name="work", bufs=2) as wpool,
            tc.tile_pool(name="fin", bufs=1) as fpool,
            tc.tile_pool(name="ps24", bufs=1, space="PSUM") as ps24pool,
            tc.tile_pool(name="psbig", bufs=2, space="PSUM") as psbig,
            tc.tile_pool(name="pssmall", bufs=2, space="PSUM") as pssm,
        ):
            # ---- load constants / params ----
            ident = cpool.tile_from(ident_d, force_copy=True, name="ident")
            iota8 = cpool.tile_from(iota_d, force_copy=True, name="iota8")
            idx_lo = cpool.tile_from(idx_lo_d, force_copy=True, name="idx_lo")
            idx_hi = cpool.tile_from(idx_hi_d, force_copy=True, name="idx_hi")

            wk = []
            for k in range(4):
                lo, hi = k * 128, min((k + 1) * 128, KIN)
                wkt = cpool.tile([hi - lo, 4 * H], f, name=f"wk{k}")
                nc.sync.dma_start(out=wkt, in_=w_all_d[lo:hi])
                wk.append(wkt)
            bias8 = cpool.tile_from(bias8_d, force_copy=True, name="bias8")
            awt0 = cpool.tile([128, D_ID], BF16, name="awt0")
            awt1 = cpool.tile([128, D_ID], BF16, name="awt1")
            nc.sync.dma_start(out=awt0, in_=attn_wt_d[0:128])
            nc.sync.dma_start(out=awt1, in_=attn_wt_d[128:256])
            post = cpool.tile_from(post_d, force_copy=True, name="post")
            wh = []
            for k in range(3):
                wht = cpool.tile([128, 2], BF16, name=f"wh{k}")
                nc.sync.dma_start(out=wht, in_=w_heads_d[k])
                wh.append(wht)
            c0t = []
            for k in range(2):
                c0k = cpool.tile([128, BC], f, name=f"c0t{k}")
                nc.sync.dma_start(out=c0k, in_=c0t_d[k * 128 : (k + 1) * 128])
                c0t.append(c0k)
            lengths = cpool.tile_from(lengths_d, force_copy=True, name="lengths")
            rbias = cpool.tile_from(rbias_d, force_copy=True, name="rbias")
            obj_lo = cpool.tile_from(obj_lo_d, force_copy=True, name="obj_lo")
            obj_hi = cpool.tile_from(obj_hi_d, force_copy=True, name="obj_hi")
            szi_sb = cpool.tile_from(size_idx_d, force_copy=True, name="szi_sb")

            # histT storage: per H-tile, [128, BC, T] bf16; history fills 0..T-2
            hist = []
            for k in range(2):
                hk = cpool.tile([128, BC, T], BF16, name=f"hist{k}")
                nc.sync.dma_start(
                    out=hk[:, :, 0 : T - 1], in_=histt_d[k * 128 : (k + 1) * 128]
                )
                hist.append(hk)

            # ---- big transposing gathers: q^T for all local batches ----
            qt_lo = cpool.tile([128, 1, NIDX], BF16, name="qt_lo")
            nc.gpsimd.dma_gather(
                out_ap=qt_lo,
                in_ap=t_lo,
                idxs_ap=idx_lo,
                num_idxs=NIDX,
                num_idxs_reg=NIDX,
                elem_size=D_ID,
                transpose=True,
                single_packet=False,
            )
            qt_hi = cpool.tile([128, 1, NIDX], BF16, name="qt_hi")
            nc.gpsimd.dma_gather(
                out_ap=qt_hi,
                in_ap=t_hi,
                idxs_ap=idx_hi,
                num_idxs=NIDX,
                num_idxs_reg=NIDX,
                elem_size=D_ID,
                transpose=True,
                single_packet=False,
            )

            # ---- LSTM single step ----
            id_lo = cpool.tile([BC, D_ID], BF16, name="id_lo")
            nc.gpsimd.indirect_dma_start(
                out=id_lo,
                out_offset=None,
                in_=t_lo,
                in_offset=IndirectOffsetOnAxis(ap=obj_lo[:, 0:1], axis=0),
            )
            id_hi = cpool.tile([BC, D_ID], BF16, name="id_hi")
            nc.gpsimd.indirect_dma_start(
                out=id_hi,
                out_offset=None,
                in_=t_hi,
                in_offset=IndirectOffsetOnAxis(ap=obj_hi[:, 0:1], axis=0),
            )
            id_sum = cpool.tile([BC, D_ID], f, name="id_sum")
            nc.vector.tensor_tensor(out=id_sum, in0=id_lo, in1=id_hi, op=OP.add)
            sz_rows = cpool.tile([BC, D_SZ], f, name="sz_rows")
            nc.gpsimd.indirect_dma_start(
                out=sz_rows,
                out_offset=None,
                in_=t_sz,
                in_offset=IndirectOffsetOnAxis(ap=szi_sb[:, 0:1], axis=0),
            )
            # transpose x -> K-major rhs tiles for the gate matmuls
            ps_xt0 = pssm.tile([128, BC], f, space="PSUM", tag="small", name="ps_xt0")
            nc.tensor.transpose(out=ps_xt0, in_=id_sum, identity=ident[0:BC, 0:BC])
            ps_xt1 = pssm.tile([D_SZ, BC], f, space="PSUM", tag="small", name="ps_xt1")
            nc.tensor.transpose(out=ps_xt1, in_=sz_rows, identity=ident[0:BC, 0:BC])
            xh0 = cpool.tile([128, BC], f, name="xh0")
            nc.vector.tensor_copy(out=xh0, in_=ps_xt0)
            xh1 = cpool.tile([128, BC], f, name="xh1")
            nc.vector.tensor_copy(out=xh1[0:D_SZ], in_=ps_xt1)
            nc.sync.dma_start(out=xh1[D_SZ:128], in_=h0t_d[0 : 128 - D_SZ])
            xh2 = cpool.tile([128, BC], f, name="xh2")
            nc.sync.dma_start(out=xh2, in_=h0t_d[128 - D_SZ : 256 - D_SZ])
            xh3 = cpool.tile([D_SZ, BC], f, name="xh3")
            nc.sync.dma_start(out=xh3, in_=h0t_d[256 - D_SZ : 256])
            xh = [xh0, xh1, xh2, xh3]

            gsb = []  # gate pre-activations, 8 M-tiles of (128, BC)
            for m in range(8):
                pg = pssm.tile([128, BC], f, space="PSUM", tag="small", name=f"pg{m}")
                for k in range(4):
                    nc.tensor.matmul(
                        out=pg,
                        lhsT=wk[k][:, m * 128 : (m + 1) * 128],
                        rhs=xh[k],
                        start=(k == 0),
                        stop=(k == 3),
                    )
                gt = cpool.tile([128, BC], f, name=f"g{m}")
                # sigmoid for i,f,o tiles; tanh for g tiles; bias fused
                func = AF.Tanh if m in (4, 5) else AF.Sigmoid
                nc.scalar.activation(
                    out=gt, in_=pg, func=func, bias=bias8[:, m : m + 1]
                )
                gsb.append(gt)

            for ht in range(2):
                t1 = cpool.tile([128, BC], f, name=f"t1_{ht}")
                nc.vector.tensor_tensor(
                    out=t1, in0=gsb[2 + ht], in1=c0t[ht], op=OP.mult
                )
                t2 = cpool.tile([128, BC], f, name=f"t2_{ht}")
                nc.vector.tensor_tensor(
                    out=t2, in0=gsb[0 + ht], in1=gsb[4 + ht], op=OP.mult
                )
                cst = cpool.tile([128, BC], f, name=f"c_{ht}")
                nc.vector.tensor_tensor(out=cst, in0=t1, in1=t2, op=OP.add)
                tct = cpool.tile([128, BC], f, name=f"tc_{ht}")
                nc.scalar.activation(out=tct, in_=cst, func=AF.Tanh)
                # h tile written straight into the last history column (bf16)
                nc.vector.tensor_tensor(
                    out=hist[ht][:, :, T - 1 : T], in0=gsb[6 + ht], in1=tct, op=OP.mult
                )

            # ---- persistent head-reduction accumulator ----
            psum24 = ps24pool.tile([24, L], f, space="PSUM", name="psum24")

            # ---- per-batch attention ----
            for b in range(BC):
                # A_b = attn_W @ histT_b  (D_ID x T)
                ps_a = pssm.tile([128, T], f, space="PSUM", tag="small", name="ps_a")
                nc.tensor.matmul(
                    out=ps_a, lhsT=awt0, rhs=hist[0][:, b, :], start=True, stop=False
                )
                nc.tensor.matmul(
                    out=ps_a, lhsT=awt1, rhs=hist[1][:, b, :], start=False, stop=True
                )
                a_sb = wpool.tile([128, T], BF16, name="a_sb", tag="a_sb")
                nc.vector.tensor_copy(out=a_sb, in_=ps_a)

                # scores_b^T (T x L) = A^T-contraction with q^T; lo+hi PSUM-fused
                ps_s = psbig.tile([128, L], f, space="PSUM", tag="scores", name="ps_s")
                for half in range(2):
                    sl = slice(half * 512, (half + 1) * 512)
                    gsl = slice(b * L + half * 512, b * L + (half + 1) * 512)
                    nc.tensor.matmul(
                        out=ps_s[:, sl], lhsT=a_sb, rhs=qt_lo[:, 0, gsl],
                        start=True, stop=False,
                    )
                    nc.tensor.matmul(
                        out=ps_s[:, sl], lhsT=a_sb, rhs=qt_hi[:, 0, gsl],
                        start=False, stop=True,
                    )
                e_sb = wpool.tile([128, L], BF16, name="e_sb", tag="e_sb")
                nc.scalar.activation(out=e_sb, in_=ps_s, func=AF.Exp)

                # sv_b, rv_b (T x 2): head projections of [hist | pos] values
                ps_w = pssm.tile([T, 2], f, space="PSUM", tag="small", name="ps_w")
                nc.tensor.matmul(
                    out=ps_w, lhsT=hist[0][:, b, :], rhs=wh[0], start=True, stop=False
                )
                nc.tensor.matmul(
                    out=ps_w, lhsT=hist[1][:, b, :], rhs=wh[1], start=False, stop=False
                )
                nc.tensor.matmul(
                    out=ps_w, lhsT=post, rhs=wh[2], start=False, stop=True
                )
                svr = wpool.tile([T, 24], BF16, name="svr", tag="svr")
                nc.vector.memset(svr, 0.0)
                nc.vector.tensor_copy(out=svr[:, b : b + 1], in_=ps_w[:, 0:1])
                nc.vector.tensor_copy(out=svr[:, 8 + b : 9 + b], in_=ps_w[:, 1:2])
                nc.vector.memset(svr[:, 16 + b : 17 + b], 1.0)

                # accumulate [N1; N2; D] rows: psum24 += svr^T @ e
                for half in range(2):
                    sl = slice(half * 512, (half + 1) * 512)
                    nc.tensor.matmul(
                        out=psum24[:, sl],
                        lhsT=svr,
                        rhs=e_sb[:, sl],
                        start=(b == 0),
                        stop=(b == BC - 1),
                        skip_group_check=True,
                    )

            # ---- heads + masked softmax over L ----
            r24 = fpool.tile([24, L], f, name="r24")
            nc.vector.tensor_copy(out=r24, in_=psum24)

            # partition-shift the three 8-row blocks down to partitions 0-7
            n1_sb = fpool.tile([BC, L], f, name="n1_sb")
            nc.sync.dma_start(out=n1_sb, in_=r24[0:8])
            n2_sb = fpool.tile([BC, L], f, name="n2_sb")
            nc.sync.dma_start(out=n2_sb, in_=r24[8:16])
            d_sb = fpool.tile([BC, L], f, name="d_sb")
            nc.sync.dma_start(out=d_sb, in_=r24[16:24])

            dinv = fpool.tile([BC, L], f, name="dinv")
            nc.vector.reciprocal(out=dinv, in_=d_sb)
            s_sb = fpool.tile([BC, L], f, name="s_sb")
            nc.vector.tensor_tensor(out=s_sb, in0=n1_sb, in1=dinv, op=OP.mult)
            e2 = fpool.tile([BC, L], f, name="e2")
            nc.scalar.activation(out=e2, in_=s_sb, func=AF.Exp)

            ru0 = fpool.tile([BC, L], f, name="ru0")
            nc.vector.tensor_tensor(out=ru0, in0=n2_sb, in1=dinv, op=OP.mult)
            ru = fpool.tile([BC, L], f, name="ru")
            nc.scalar.activation(out=ru, in_=ru0, func=AF.Identity, bias=rbias)

            valid = fpool.tile([BC, 1], f, name="valid")
            nc.vector.tensor_scalar_max(out=valid, in0=lengths, scalar1=1.0)
            me = fpool.tile([BC, L], f, name="me")
            nc.vector.scalar_tensor_tensor(
                out=me, in0=iota8, scalar=valid, in1=e2, op0=OP.is_lt, op1=OP.mult
            )
            den = fpool.tile([BC, 1], f, name="den")
            nc.vector.tensor_reduce(
                out=den, in_=me, axis=mybir.AxisListType.X, op=OP.add
            )
            dinv2 = fpool.tile([BC, 1], f, name="dinv2")
            nc.vector.reciprocal(out=dinv2, in_=den)
            probs = fpool.tile([BC, L], f, name="probs")
            nc.vector.tensor_scalar_mul(out=probs, in0=me, scalar1=dinv2)

            nc.sync.dma_start(out=out_probs, in_=probs)
            nc.sync.dma_start(out=out_reuse, in_=ru)

    nc.compile()
    return nc


def _wrap_idxs(idx):
    """idx (N,) -> (128, N//16) int16 wrapped-16 + replicated layout."""
    n = len(idx)
    cols = idx.reshape(n // 16, 16).T.astype(np.int16)  # (16, n/16)
    return np.tile(cols, (8, 1))


def make_in_maps(inputs):
    """Host-side prep: shard over batch, transpose params, build per-core maps."""
    inp = {k: np.asarray(v) for k, v in inputs.items()}

    table = inp["obj_id_table"].astype(np.float32)

    w_all = np.concatenate(
        [inp["W_ih"].T.astype(np.float32), inp["W_hh"].T.astype(np.float32)], axis=0
    )  # (448, 1024)
    bias = (inp["b_ih"] + inp["b_hh"]).astype(np.float32)  # (1024,)
    bias8 = np.ascontiguousarray(bias.reshape(8, 128).T)  # (128, 8) col m = tile m
    attn_wt = np.ascontiguousarray(inp["attn_W"].T.astype(BF16_NP))  # (256, 128)
    post = np.ascontiguousarray(inp["pos_table"].T.astype(BF16_NP))  # (128, 128)
    scorer = inp["scorer_W"][0].astype(np.float32)  # (384,)
    reuse = inp["reuse_W"][0].astype(np.float32)
    w_heads = np.zeros((3, 128, 2), np.float32)
    w_heads[0, :, 0], w_heads[0, :, 1] = scorer[0:128], reuse[0:128]
    w_heads[1, :, 0], w_heads[1, :, 1] = scorer[128:256], reuse[128:256]
    w_heads[2, :, 0], w_heads[2, :, 1] = scorer[256:384], reuse[256:384]

    shared = {
        "t_sz": np.ascontiguousarray(inp["obj_size_table"].astype(np.float32)),
        "w_all": w_all,
        "bias8": bias8,
        "attn_wt": attn_wt,
        "post": post,
        "w_heads": w_heads.astype(BF16_NP),
    }

    in_maps = []
    for c in range(NCORES):
        sl = slice(c * BC, (c + 1) * BC)
        cache = inp["cache_lines"][sl].astype(np.int64)  # (BC, L)
        obj = inp["obj_id"][sl].astype(np.int64)  # (BC,)

        used = np.unique(np.concatenate([cache.reshape(-1), obj]))
        used_lo = set(used[used < VH].tolist())
        used_hi = set((used[used >= VH] - VH).tolist())
        v = next(i for i in range(VH) if i not in used_lo)
        w2 = next(i for i in range(VH) if i not in used_hi)

        tl = table[0:VH].astype(BF16_NP)
        tl[v] = 0
        th = table[VH:V_ID].astype(BF16_NP)
        th[w2] = 0

        flat = cache.reshape(-1)
        ilo = np.where(flat < VH, flat, v)
        ihi = np.where(flat >= VH, flat - VH, w2)

        m = dict(shared)
        m["t_lo"] = np.ascontiguousarray(tl)
        m["t_hi"] = np.ascontiguousarray(th)
        m["idx_lo"] = _wrap_idxs(ilo)
        m["idx_hi"] = _wrap_idxs(ihi)
        m["obj_lo"] = np.where(obj < VH, obj, v).astype(np.int32).reshape(BC, 1)
        m["obj_hi"] = (
            np.where(obj >= VH, obj - VH, w2).astype(np.int32).reshape(BC, 1)
        )
        m["size_idx"] = np.ascontiguousarray(
            inp["obj_size"][sl].astype(np.int32).reshape(BC, 1)
        )
        m["lengths_f"] = np.ascontiguousarray(
            inp["lengths"][sl].astype(np.float32).reshape(BC, 1)
        )
        m["c0t"] = np.ascontiguousarray(inp["c0"][sl].T.astype(np.float32))
        m["h0t"] = np.ascontiguousarray(inp["h0"][sl].T.astype(np.float32))
        m["histt"] = np.ascontiguousarray(
            inp["history"][sl].transpose(2, 0, 1).astype(BF16_NP)
        )
        m["rbias"] = np.full((BC, 1), float(inp["reuse_b"][0]), np.float32)
        in_maps.append(m)
    return in_maps


_PROGRAM = None


def get_program():
    global _PROGRAM
    if _PROGRAM is None:
        _PROGRAM = build_program()
    return _PROGRAM


def kernel(**inputs):
    nc = get_program()
    in_maps = make_in_maps(inputs)
    res = run_bass_kernel_spmd(nc, in_maps, core_ids=list(range(NCORES)))
    probs = np.concatenate([res.results[c]["out_probs"] for c in range(NCORES)], 0)
    reuse = np.concatenate([res.results[c]["out_reuse"] for c in range(NCORES)], 0)
    return probs, reuse


if __name__ == "__main__":
    get_program()
    print("program built OK")
